# revision 3
# baseline (speedup 1.0000x reference)
"""Distributed 1D attention kernel for Trainium2 (8 NeuronCores).

Problem: x [4,256,2048], y [4,256,2048] ->
  q = Wq@x, k = Wk@y, v = Wv@y  (per-head d=128, H=8 heads)
  out = Wo @ concat_h(softmax(q^T k / sqrt(128)) applied to v)   -> [4,128,2048]

Sharding: core = 2*b + g where b in [0,4) is the batch and g in {0,1} picks
heads [4g, 4g+4). Each core computes its 4 (b,h) attention pairs plus the
partial Wo projection for its head group; the host sums the two partials
per batch.

Device-side schedule (per core), 3-deep software pipeline over 16 slots
(slot = (head h, x-block of 512)):
  A(s):  logitsT tiles [y=128p, 1024] = matmul(lhsT=K_h[d, ytile], rhs=Q_h[d, xblk])
         + exp on ScalarE (PSUM->SBUF bf16, scale folded)        [window s]
  B1(s): 8 pair-sums S[g] = E[2g]+E[2g+1], split 4 on GPSIMD + 4 on DVE
                                                                  [window s+1]
  B2(s): AV accumulation (16 matmuls), denominator tree -> 2 tiles ->
         2 ones-matmuls, Newton reciprocal on DVE (bit-trick seed + 1 NR,
         bf16), normalize; Wo partial at h==3                     [window s+2]

The denominator tree + Newton recip replace the baseline's 8 ones-matmuls
and the DVE iterative-divide RECIPROCAL (8 cyc/elem), moving PE and DVE
work onto the otherwise-idle GPSIMD so the ScalarE exp stream (~147us)
sets the pace. Projections are interleaved into PE slack; DMAs are split
so the first attention slot starts ~6us in.
"""

import sys

if "/opt/trn_rl_repo" not in sys.path:
    sys.path.insert(0, "/opt/trn_rl_repo")

import numpy as np
import ml_dtypes


def _install_ntff_shim():
    """antenv.axon_hooks is absent from this image, which crashes
    run_bass_kernel_spmd(trace=True). Recreate it from the hook factory
    that trn_agent_boot ships."""
    import types

    if "antenv.axon_hooks" in sys.modules:
        return
    mod = types.ModuleType("antenv.axon_hooks")
    _hook = [None]
    mod.set_axon_ntff_profile_hook = lambda h: _hook.__setitem__(0, h)
    mod.get_axon_ntff_profile_hook = lambda: _hook[0]
    sys.modules["antenv.axon_hooks"] = mod
    try:
        import antenv

        antenv.axon_hooks = mod
    except ImportError:
        pass
    try:
        from trn_agent_boot.trn_boot import _ntff_profile_via_ctypes

        mod.set_axon_ntff_profile_hook(
            _ntff_profile_via_ctypes("/opt/axon/libaxon_pjrt.so")
        )
    except Exception:
        pass


_install_ntff_shim()

import concourse.bass as bass
import concourse.mybir as mybir
import concourse.tile as tile
from concourse.bass_utils import run_bass_kernel_spmd

B, C, N, H, D = 4, 256, 2048, 8, 128
HPC = H // 2  # heads per core
NCORES = 8
BF = mybir.dt.bfloat16
F32 = mybir.dt.float32
I16 = mybir.dt.int16
NYT = N // 128  # 16 y tiles
NXB = N // 512  # 4 x blocks
SCALE = 1.0 / float(np.sqrt(D))
MAGIC = 0x7EF1  # bf16 reciprocal bit-trick constant

LAST_EXEC_NS = None
LAST_RESULTS = None


def _split_multi_waits(nc):
    """This walrus build accepts at most ONE sync wait per instruction;
    Tile's semaphore assignment attaches several. Hoist the extras into
    standalone event-semaphore instructions on the same engine."""
    ctr = 0
    for fn in nc.m.functions:
        for blk in fn.blocks:
            new_list = []
            changed = False
            for inst in blk.instructions:
                si = inst.sync_info
                if si is not None and len(si.on_wait) > 1:
                    waits = list(si.on_wait)
                    ups = list(si.on_update)
                    for w in waits[:-1]:
                        ev = mybir.InstEventSemaphore(
                            name=f"waitsplit-{ctr}", ins=[], outs=[]
                        )
                        ctr += 1
                        ev.engine = inst.engine
                        ev.sync_info = mybir.SyncInfo(on_wait=[w], on_update=[])
                        new_list.append(ev)
                    inst.sync_info = mybir.SyncInfo(on_wait=[waits[-1]], on_update=ups)
                    changed = True
                new_list.append(inst)
            if changed:
                blk.instructions = new_list
    return ctr


def _build_nc():
    nc = bass.Bass("TRN2", target_bir_lowering=False, debug=False)

    xb = nc.dram_tensor("xb", [C, N], BF, kind="ExternalInput")
    yb = nc.dram_tensor("yb", [C, N], BF, kind="ExternalInput")
    # wpack = [WKT | WQT | WVT] along the output dim, [c, 3*hd]
    wpack = nc.dram_tensor("wpack", [C, 3 * HPC * D], BF, kind="ExternalInput")
    wot = nc.dram_tensor("wot", [HPC * D, D], BF, kind="ExternalInput")  # [hd, o]
    out = nc.dram_tensor("out", [D, N], F32, kind="ExternalOutput")

    EXPF = mybir.ActivationFunctionType.Exp
    ADD = mybir.AluOpType.add
    XOR = mybir.AluOpType.bitwise_xor
    SUB = mybir.AluOpType.subtract
    MUL = mybir.AluOpType.mult

    with tile.TileContext(nc) as tc:
        with (
            tc.tile_pool(name="w", bufs=1) as wpool,
            tc.tile_pool(name="big", bufs=1) as bigpool,
            tc.tile_pool(name="e", bufs=24) as epool,
            tc.tile_pool(name="s", bufs=20) as spool,
            tc.tile_pool(name="uw", bufs=12) as uwpool,
            tc.tile_pool(name="nwt", bufs=10) as nwtpool,
            tc.tile_pool(name="att", bufs=4) as attpool,
            tc.tile_pool(name="small", bufs=4) as smallpool,
            tc.tile_pool(name="pl", bufs=2, space="PSUM") as plpool,
            tc.tile_pool(name="po", bufs=2, space="PSUM") as popool,
            tc.tile_pool(name="pd", bufs=2, space="PSUM") as pdpool,
        ):
            # ---- constants + ACT exp-table preload at t=0 -------------------
            ONES = wpool.tile([128, 128], BF, tag="ONES")
            nc.gpsimd.memset(ONES[:], 1.0)
            TMP = wpool.tile([128, 32], BF, tag="TMP")
            nc.gpsimd.memset(TMP[:, 0:16], 0.0)
            # loads the exp_and_others table set (~2.7us) while DMAs run
            nc.scalar.activation(TMP[:, 16:32], TMP[:, 0:16], EXPF)

            # ---- input loads, split so h0's K/Q projections start early ----
            xr = xb.rearrange("(kt p) n -> p kt n", p=128)
            yr = yb.rearrange("(kt p) n -> p kt n", p=128)
            wpr = wpack.rearrange("(kt p) m -> p kt m", p=128)
            WP = wpool.tile([128, 2, 3 * HPC * D], BF, tag="WP")
            nc.sync.dma_start(WP[:, :, 0 : HPC * D], wpr[:, :, 0 : HPC * D])  # WKT
            Y = bigpool.tile([128, 2, N], BF, tag="Y")
            nc.sync.dma_start(Y[:], yr[:, :, :])
            nc.sync.dma_start(
                WP[:, :, HPC * D : 3 * HPC * D], wpr[:, :, HPC * D : 3 * HPC * D]
            )
            X = bigpool.tile([128, 2, N], BF, tag="X")
            nc.sync.dma_start(X[:, :, 0:512], xr[:, :, 0:512])
            nc.sync.dma_start(X[:, :, 512:N], xr[:, :, 512:N])
            WOT = wpool.tile([128, HPC, D], BF, tag="WOT")
            nc.sync.dma_start(WOT[:], wot.rearrange("(h p) o -> p h o", p=128))
            WKT = WP[:, :, 0 : HPC * D]
            WQT = WP[:, :, HPC * D : 2 * HPC * D]
            WVT = WP[:, :, 2 * HPC * D : 3 * HPC * D]

            # HAM warm-up: keep the PE clock-gate open while input DMAs run,
            # so the first real matmuls start at 2.4 GHz instead of 1.2.
            WARM = plpool.tile([128, 1024], F32, tag="pl", name="warm")
            for _wi in range(44):
                nc.tensor.matmul(
                    WARM[:, :128], ONES[:], ONES[:], start=True, stop=True
                )

            # ---- projections ------------------------------------------------
            Q = bigpool.tile([128, HPC, N], BF, tag="Q")
            K = bigpool.tile([128, HPC, N], BF, tag="K")
            VT = bigpool.tile([128, NYT, HPC * D], BF, tag="VT")

            def proj_k(h):
                hs = slice(h * 128, (h + 1) * 128)
                for nb in range(NXB):
                    ns = slice(nb * 512, (nb + 1) * 512)
                    pk = pdpool.tile([128, 512], F32, tag="pd", name=f"pk_{h}_{nb}")
                    nc.tensor.matmul(
                        pk[:], WKT[:, 0, hs], Y[:, 0, ns], start=True, stop=False
                    )
                    nc.tensor.matmul(
                        pk[:], WKT[:, 1, hs], Y[:, 1, ns], start=False, stop=True
                    )
                    nc.vector.tensor_copy(K[:, h, ns], pk[:])

            def proj_q(h, nb0, nb1):
                hs = slice(h * 128, (h + 1) * 128)
                for nb in range(nb0, nb1):
                    ns = slice(nb * 512, (nb + 1) * 512)
                    ps = popool.tile([128, 512], F32, tag="po", name=f"pq_{h}_{nb}")
                    nc.tensor.matmul(
                        ps[:], WQT[:, 0, hs], X[:, 0, ns], start=True, stop=False
                    )
                    nc.tensor.matmul(
                        ps[:], WQT[:, 1, hs], X[:, 1, ns], start=False, stop=True
                    )
                    nc.vector.tensor_copy(Q[:, h, ns], ps[:])

            def proj_v(yt0, yt1):
                for yt in range(yt0, yt1):
                    ys = slice(yt * 128, (yt + 1) * 128)
                    pv = popool.tile([128, 512], F32, tag="po", name=f"pv_{yt}")
                    nc.tensor.matmul(
                        pv[:], Y[:, 0, ys], WVT[:, 0, :], start=True, stop=False
                    )
                    nc.tensor.matmul(
                        pv[:], Y[:, 1, ys], WVT[:, 1, :], start=False, stop=True
                    )
                    nc.vector.tensor_copy(VT[:, yt, :], pv[:])

            # ---- attention: slots (h, xblk), 3-deep pipeline ---------------
            slots = [(h, xblk) for h in range(HPC) for xblk in range(NXB)]
            E_t = {}
            S_t = {}
            W_t = {}
            att_tiles = {}

            def phase_a(s):
                h, xblk = slots[s]
                xs = slice(xblk * 512, (xblk + 1) * 512)
                E = [
                    epool.tile([128, 2, 512], BF, tag="E", name=f"E_{s}_{g}")
                    for g in range(8)
                ]
                for g in range(8):
                    pl = plpool.tile([128, 1024], F32, tag="pl", name=f"pl_{s}_{g}")
                    for j in range(2):
                        yt = 2 * g + j
                        nc.tensor.matmul(
                            pl[:, j * 512 : (j + 1) * 512],
                            K[:, h, yt * 128 : (yt + 1) * 128],
                            Q[:, h, xs],
                            start=True,
                            stop=True,
                        )
                    nc.scalar.activation(E[g][:], pl[:], EXPF, scale=SCALE)
                E_t[s] = E

            def phase_b1(s):
                # pair-sums: S[g] = E[g][:,0,:] + E[g][:,1,:]
                # even g on GPSIMD (otherwise idle), odd g on DVE
                E = E_t[s]
                S = [
                    spool.tile([128, 512], BF, tag="S", name=f"S_{s}_{g}")
                    for g in range(8)
                ]
                for g in range(8):
                    eng = nc.gpsimd if g % 2 == 0 else nc.vector
                    eng.tensor_add(S[g][:], E[g][:, 0, :], E[g][:, 1, :])
                S_t[s] = S

            def phase_b2(s):
                h, xblk = slots[s]
                E = E_t.pop(s)
                S = S_t.pop(s)
                # AV first: only needs E, keeps PE busy while the tree runs
                po = popool.tile([128, 512], F32, tag="po", name=f"pav_{s}")
                for g in range(8):
                    for j in range(2):
                        yt = 2 * g + j
                        nc.tensor.matmul(
                            po[:],
                            VT[:, yt, h * 128 : (h + 1) * 128],
                            E[g][:, j, :],
                            start=(yt == 0),
                            stop=(yt == NYT - 1),
                        )
                # denominator tree on DVE: 8 -> 4 -> 2 tiles
                U = [
                    uwpool.tile([128, 512], BF, tag="UW", name=f"U_{s}_{i}")
                    for i in range(4)
                ]
                for i in range(4):
                    nc.vector.tensor_add(U[i][:], S[2 * i][:], S[2 * i + 1][:])
                W = [
                    uwpool.tile([128, 512], BF, tag="UW", name=f"W_{s}_{i}")
                    for i in range(2)
                ]
                for i in range(2):
                    nc.vector.tensor_add(W[i][:], U[2 * i][:], U[2 * i + 1][:])
                pd = pdpool.tile([128, 512], F32, tag="pd", name=f"pden_{s}")
                nc.tensor.matmul(pd[:], ONES[:], W[0][:], start=True, stop=False)
                nc.tensor.matmul(pd[:], ONES[:], W[1][:], start=False, stop=True)
                # Newton reciprocal: rc ~= 1/den in bf16
                Dt = nwtpool.tile([128, 512], BF, tag="nwt", name=f"D_{s}")
                nc.vector.tensor_copy(Dt[:], pd[:])
                r0 = nwtpool.tile([128, 512], BF, tag="nwt", name=f"r0_{s}")
                # r0 bits = MAGIC - D bits  ==  (D - MAGIC) * -1, all-arith int16
                nc.vector.tensor_scalar(
                    r0[:].bitcast(I16), Dt[:].bitcast(I16), MAGIC, -1, SUB, MUL
                )
                tt = nwtpool.tile([128, 512], BF, tag="nwt", name=f"t_{s}")
                nc.vector.tensor_tensor(tt[:], Dt[:], r0[:], MUL)
                uu = nwtpool.tile([128, 512], BF, tag="nwt", name=f"u_{s}")
                # u = 2 - t  ==  (t - 2) * -1
                nc.vector.tensor_scalar(uu[:], tt[:], 2.0, -1.0, SUB, MUL)
                rc = nwtpool.tile([128, 512], BF, tag="nwt", name=f"rc_{s}")
                nc.vector.tensor_tensor(rc[:], r0[:], uu[:], MUL)
                # normalize into the ATT tile for this xblk
                if h == 0:
                    att_tiles[xblk] = attpool.tile(
                        [128, HPC, 512], BF, tag="ATT", name=f"ATT_{xblk}"
                    )
                ATT = att_tiles[xblk]
                nc.vector.tensor_tensor(ATT[:, h, :], po[:], rc[:], MUL)
                if h == HPC - 1:
                    xs = slice(xblk * 512, (xblk + 1) * 512)
                    pw = pdpool.tile([128, 512], F32, tag="pd", name=f"pw_{xblk}")
                    for hh in range(HPC):
                        nc.tensor.matmul(
                            pw[:],
                            WOT[:, hh, :],
                            ATT[:, hh, :],
                            start=(hh == 0),
                            stop=(hh == HPC - 1),
                        )
                    ob = smallpool.tile([128, 512], F32, tag="osb", name=f"ob_{xblk}")
                    nc.vector.tensor_copy(ob[:], pw[:])
                    nc.sync.dma_start(out[:, xs], ob[:])

            proj_k(0)
            proj_q(0, 0, NXB)
            for s in range(len(slots)):
                phase_a(s)
                if s == 0:
                    proj_v(0, 8)
                elif s == 1:
                    proj_v(8, NYT)
                elif s == 2:
                    proj_k(1)
                    proj_q(1, 0, 1)
                elif s == 4:
                    proj_q(1, 1, NXB)
                elif s == 6:
                    proj_k(2)
                    proj_q(2, 0, 1)
                elif s == 8:
                    proj_q(2, 1, NXB)
                elif s == 10:
                    proj_k(3)
                    proj_q(3, 0, 1)
                elif s == 12:
                    proj_q(3, 1, NXB)
                if s >= 1:
                    phase_b1(s - 1)
                if s >= 2:
                    phase_b2(s - 2)
            phase_b1(len(slots) - 1)
            phase_b2(len(slots) - 2)
            phase_b2(len(slots) - 1)

    _split_multi_waits(nc)
    return nc


_NC = None


def _get_nc():
    global _NC
    if _NC is None:
        _NC = _build_nc()
    return _NC


def kernel(x, y, Wq, Wk, Wv, Wo):
    global LAST_EXEC_NS, LAST_RESULTS
    x = np.asarray(x, dtype=np.float32)
    y = np.asarray(y, dtype=np.float32)
    Wq3 = np.asarray(Wq, dtype=np.float32).reshape(H, D, C)
    Wk3 = np.asarray(Wk, dtype=np.float32).reshape(H, D, C)
    Wv3 = np.asarray(Wv, dtype=np.float32).reshape(H, D, C)
    Wo2 = np.asarray(Wo, dtype=np.float32)  # [D, H*D]

    bf16 = ml_dtypes.bfloat16

    in_maps = []
    for core in range(NCORES):
        b, g = core // 2, core % 2
        hsl = slice(4 * g, 4 * g + HPC)
        wqt = Wq3[hsl].reshape(HPC * D, C).T  # [c, hd]
        wkt = Wk3[hsl].reshape(HPC * D, C).T
        wvt = Wv3[hsl].reshape(HPC * D, C).T
        wot = Wo2[:, 4 * g * D : (4 * g + HPC) * D].T  # [hd, o]
        wpack = np.concatenate([wkt, wqt, wvt], axis=1)  # [c, 3*hd]
        in_maps.append(
            {
                "xb": np.ascontiguousarray(x[b]).astype(bf16),
                "yb": np.ascontiguousarray(y[b]).astype(bf16),
                "wpack": np.ascontiguousarray(wpack).astype(bf16),
                "wot": np.ascontiguousarray(wot).astype(bf16),
            }
        )

    import os

    trace = bool(int(os.environ.get("ATTN_TRACE", "0")))
    res = run_bass_kernel_spmd(
        _get_nc(), in_maps, core_ids=list(range(NCORES)), trace=trace
    )
    LAST_EXEC_NS = res.exec_time_ns
    LAST_RESULTS = res

    out = np.empty((B, D, N), dtype=np.float32)
    for b in range(B):
        out[b] = res.results[2 * b]["out"] + res.results[2 * b + 1]["out"]
    return out
